# revision 15
# baseline (speedup 1.0000x reference)
"""CKA (centered kernel alignment) on 8 Trainium2 NeuronCores.

Math: for G = x @ x.T, centering H G H (H = I - 11^T/n) satisfies
H G H = (Hx)(Hx)^T, so with xc = x - colmean(x):
    (K * L).sum() = ||xc^T @ yc||_F^2
and xc^T yc = x^T y - (1/n) sx sy^T  (sx/sy = column sums).
So CKA reduces to small feature-covariance matmuls instead of
8192x8192 Gram matrices (~120 GFLOP instead of ~412 + 536MB of traffic).

Sharding: rows (n) split across 8 cores. Each core computes partial
covariances (contraction over its 1024 rows) in bf16 (validated:
rel-err ~1e-3 on the final scalar incl. the bf16 CCE ring), packs
partials + column-sum partials into DRAM buffers, ReduceScatters them,
applies the exact rank-1 centering correction to its reduced chunk,
squares and reduces. Host sums 8 tiny [128,16] partials and applies the
final scalar formula.

Phase order is chosen so each ReduceScatter overlaps the next compute
phase. Collectives starve the kernel's own DMA queues (shared SDMA
hardware), so the spill pool is sized to hold an entire phase of
PSUM spills in SBUF — the PE keeps streaming while spill DMAs crawl
during a collective and burst afterwards:

  s-sums -> Cxx h1 -> [RS(bufA) || Cxy+Cyy] -> [RS(buf1) || Cxx h2]
         -> RS(bufB) -> center/square/reduce

Chunk layouts (per chunk c of 8):
  buf1 chunk (195 rows x 2048, bf16):
    rows   0..127 : Cxy[128c:+128, 0:1024] | Cxy[1024+128c:+128, 0:1024]
    rows 128..191 : Cyy[64c:+64, 0:1024]   | Cyy[512+64c:+64, 0:1024]
    row  192      : sx (full 2048 col-sums of x, replicated per chunk)
    row  193      : sy (full 1024 col-sums of y) | junk
    row  194      : packed u = s/n slices for THIS chunk's rows:
                    [sx[128c:+128]/n | sx[1024+128c:+128]/n
                     | sy[64c:+64]/n | sy[512+64c:+64]/n | junk]
  bufA chunk (128 rows): Cxx[128c:+128, :]       (Cxx rows 0..1023)
  bufB chunk (128 rows): Cxx[1024+128c:+128, :]  (Cxx rows 1024..2047)

Replicating the per-chunk s-slices inside each chunk keeps the SPMD
program rank-uniform: every core reads its u/v vectors at the same
static offsets of its own reduced chunk.
"""

import numpy as np

N_CORES = 8
N = 8192
NS = N // N_CORES      # 1024 rows per core
DX = 2048
DY = 1024
P = 128
KT = NS // P           # 8 contraction tiles per core
INV_N = 1.0 / N
C1R = 195              # buf1 rows per chunk

_COMPILED = None


def _build():
    import concourse.bacc as bacc
    import concourse.mybir as mybir
    import concourse.tile as tile

    f32 = mybir.dt.float32
    bf16 = mybir.dt.bfloat16

    nc = bacc.Bacc("TRN2", target_bir_lowering=False, debug=False,
                   num_devices=N_CORES)
    x = nc.dram_tensor("x", [NS, DX], bf16, kind="ExternalInput")
    y = nc.dram_tensor("y", [NS, DY], bf16, kind="ExternalInput")
    out = nc.dram_tensor("partials", [P, 20], f32, kind="ExternalOutput")

    rg = [list(range(N_CORES))]

    with tile.TileContext(nc) as tc:
        with (
            tc.tile_pool(name="persist", bufs=1) as persist,
            tc.tile_pool(name="spill", bufs=4) as spill,
            tc.tile_pool(name="dram", bufs=1, space="DRAM") as dram,
        ):
            # ---------------- load (already bf16) ----------------
            xb = persist.tile([P, KT, DX], bf16)
            yb = persist.tile([P, KT, DY], bf16)
            for k in range(KT):
                nc.sync.dma_start(xb[:, k, :], x[k * P:(k + 1) * P, :])
                nc.sync.dma_start(yb[:, k, :], y[k * P:(k + 1) * P, :])

            # DRAM buffers for the collectives
            buf1 = dram.tile([C1R * N_CORES, DX], bf16)
            bufA = dram.tile([1024, DX], bf16)
            bufB1 = dram.tile([768, DX], bf16)
            bufB2 = dram.tile([256, DX], bf16)
            ch1 = dram.tile([C1R, DX], bf16)
            chA = dram.tile([P, DX], bf16)
            chB1 = dram.tile([96, DX], bf16)
            chB2 = dram.tile([32, DX], bf16)
            scr_sx = dram.tile([1, DX], bf16)
            scr_sy = dram.tile([1, DY], bf16)
            scr_ux = dram.tile([1, DX], bf16)
            scr_uy = dram.tile([1, DY], bf16)
            bv = buf1[:].rearrange("(c r) w -> c r w", r=C1R)

            # ---------------- column sums (ones-matmul) ----------------
            ones = persist.tile([P, 1], bf16)
            nc.vector.memset(ones[:], 1.0)
            with tc.tile_pool(name="psum_s", bufs=1, space="PSUM") as psum_s:
                ps_sx = psum_s.tile([1, DX], f32)
                ps_sy = psum_s.tile([1, DY], f32)
                for k in range(KT):
                    for j in range(DX // 512):
                        nc.tensor.matmul(ps_sx[0:1, j * 512:(j + 1) * 512],
                                         ones[:], xb[:, k, j * 512:(j + 1) * 512],
                                         start=(k == 0), stop=(k == KT - 1))
                    for j in range(DY // 512):
                        nc.tensor.matmul(ps_sy[0:1, j * 512:(j + 1) * 512],
                                         ones[:], yb[:, k, j * 512:(j + 1) * 512],
                                         start=(k == 0), stop=(k == KT - 1))
                sx_sb = persist.tile([1, DX], bf16)
                sy_sb = persist.tile([1, DY], bf16)
                ux_sb = persist.tile([1, DX], bf16)
                uy_sb = persist.tile([1, DY], bf16)
                nc.scalar.copy(sx_sb[:], ps_sx[:])
                nc.scalar.copy(sy_sb[:], ps_sy[:])
                nc.scalar.mul(ux_sb[:], ps_sx[:], INV_N)
                nc.scalar.mul(uy_sb[:], ps_sy[:], INV_N)

            # s vectors -> DRAM scratch -> scatter into buf1 rows
            nc.sync.dma_start(scr_sx[:], sx_sb[:])
            nc.sync.dma_start(scr_sy[:], sy_sb[:])
            nc.sync.dma_start(scr_ux[:], ux_sb[:])
            nc.sync.dma_start(scr_uy[:], uy_sb[:])
            for c in range(N_CORES):
                nc.sync.dma_start(bv[c, 192, :], scr_sx[0, :])
                nc.sync.dma_start(bv[c, 193, 0:DY], scr_sy[0, :])
            nc.sync.dma_start(
                bv[:, 194, 0:128],
                scr_ux[0:1, 0:1024].rearrange("a (c k) -> (a c) k", k=128))
            nc.sync.dma_start(
                bv[:, 194, 128:224],
                scr_ux[0:1, 1024:1792].rearrange("a (c k) -> (a c) k", k=96))
            nc.sync.dma_start(
                bv[:, 194, 224:256],
                scr_ux[0:1, 1792:2048].rearrange("a (c k) -> (a c) k", k=32))
            nc.sync.dma_start(
                bv[:, 194, 384:512],
                scr_ux[0:1, 1024:2048].rearrange("a (c k) -> (a c) k", k=128))
            nc.sync.dma_start(
                bv[:, 194, 256:320],
                scr_uy[0:1, 0:512].rearrange("a (c k) -> (a c) k", k=64))
            nc.sync.dma_start(
                bv[:, 194, 320:384],
                scr_uy[0:1, 512:1024].rearrange("a (c k) -> (a c) k", k=64))

            with tc.tile_pool(name="psum_mm", bufs=8, space="PSUM") as psum_mm:

                def cxx_half(half, dsts):
                    for mh in range(8):
                        m = half * 8 + mh
                        pss = [psum_mm.tile([P, 512], f32, tag="ps", name="ps")
                               for _ in range(4)]
                        for k in range(KT):
                            for n4 in range(4):
                                nc.tensor.matmul(
                                    pss[n4][:], xb[:, k, m * P:(m + 1) * P],
                                    xb[:, k, n4 * 512:(n4 + 1) * 512],
                                    start=(k == 0), stop=(k == KT - 1))
                        if mh < 6:
                            dst, r0 = dsts[0], mh * P
                        else:
                            dst, r0 = dsts[1], (mh - 6) * P
                        for n4 in range(4):
                            st = spill.tile([P, 512], bf16, tag="st",
                                            name="st", bufs=56)
                            nc.vector.tensor_copy(st[:], pss[n4][:])
                            nc.sync.dma_start(
                                dst[r0:r0 + P,
                                    n4 * 512:(n4 + 1) * 512], st[:])

                # ---- Cxx first half -> bufA, then its ReduceScatter ----
                cxx_half(0, (bufA, bufA[768:1024, :]))
                nc.gpsimd.collective_compute(
                    "ReduceScatter", mybir.AluOpType.add, replica_groups=rg,
                    ins=[bufA[:]], outs=[chA[:]])

                # ---- Cxy (overlaps RS(bufA)) ----
                for m in range(DX // P):
                    pss = [psum_mm.tile([P, 512], f32, tag="ps", name="ps")
                           for _ in range(2)]
                    for k in range(KT):
                        for n2 in range(2):
                            nc.tensor.matmul(
                                pss[n2][:], xb[:, k, m * P:(m + 1) * P],
                                yb[:, k, n2 * 512:(n2 + 1) * 512],
                                start=(k == 0), stop=(k == KT - 1))
                    c, col0 = (m, 0) if m < 8 else (m - 8, 1024)
                    for n2 in range(2):
                        st = spill.tile([P, 512], bf16, tag="st",
                                        name="st", bufs=56)
                        nc.vector.tensor_copy(st[:], pss[n2][:])
                        nc.sync.dma_start(
                            bv[c, 0:P, col0 + n2 * 512:col0 + (n2 + 1) * 512],
                            st[:])

                # ---- Cyy ----
                for m in range(DY // P):
                    pss = [psum_mm.tile([P, 512], f32, tag="ps", name="ps")
                           for _ in range(2)]
                    for k in range(KT):
                        for n2 in range(2):
                            nc.tensor.matmul(
                                pss[n2][:], yb[:, k, m * P:(m + 1) * P],
                                yb[:, k, n2 * 512:(n2 + 1) * 512],
                                start=(k == 0), stop=(k == KT - 1))
                    for n2 in range(2):
                        st = spill.tile([P, 512], bf16, tag="st",
                                        name="st", bufs=56)
                        nc.vector.tensor_copy(st[:], pss[n2][:])
                        for h in range(2):
                            mm = m if m < 4 else m - 4
                            c = 2 * mm + h
                            col0 = (0 if m < 4 else 1024) + n2 * 512
                            nc.sync.dma_start(
                                bv[c, 128:192, col0:col0 + 512],
                                st[h * 64:(h + 1) * 64, :])

                # ---- ReduceScatter buf1 (overlaps Cxx h2) ----
                nc.gpsimd.collective_compute(
                    "ReduceScatter", mybir.AluOpType.add, replica_groups=rg,
                    ins=[buf1[:]], outs=[ch1[:]])

                # ---- Cxx second half -> bufB, then its ReduceScatter ----
                cxx_half(1, (bufB1, bufB2))
                nc.gpsimd.collective_compute(
                    "ReduceScatter", mybir.AluOpType.add, replica_groups=rg,
                    ins=[bufB1[:]], outs=[chB1[:]])
                nc.gpsimd.collective_compute(
                    "ReduceScatter", mybir.AluOpType.add, replica_groups=rg,
                    ins=[bufB2[:]], outs=[chB2[:]])

            # ------------- stage 2: center, square, reduce -------------
            sxr = persist.tile([1, DX], bf16)
            nc.sync.dma_start(sxr[:], ch1[192:193, :])
            syr = persist.tile([1, DY], bf16)
            nc.sync.dma_start(syr[:], ch1[193:194, 0:DY])
            ur = persist.tile([1, 512], bf16)
            nc.sync.dma_start(ur[:], ch1[194:195, 0:512])
            c1a = persist.tile([P, DX], bf16)
            nc.sync.dma_start(c1a[:], ch1[0:P, :])
            c1b = persist.tile([64, DX], bf16)
            nc.sync.dma_start(c1b[:], ch1[P:192, :])
            c2a = persist.tile([P, DX], bf16)
            nc.sync.dma_start(c2a[:], chA[:])
            c2b1 = persist.tile([96, DX], bf16)
            nc.sync.dma_start(c2b1[:], chB1[:])
            c2b2 = persist.tile([32, DX], bf16)
            nc.sync.dma_start(c2b2[:], chB2[:])

            acc = persist.tile([P, 20], f32)
            nc.vector.memset(acc[:], 0.0)

            # jobs: (chunk tile, rows, list of (u, v) per 512-col quarter, col)
            # u offsets in ur: uxA@0, uxB1@128, uxB2@224, uyA@256, uyB@320
            uxA, uxB1 = ur[0:1, 0:128], ur[0:1, 128:224]
            uxB2 = ur[0:1, 224:256]
            uyA, uyB = ur[0:1, 256:320], ur[0:1, 320:384]
            sx4 = [sxr[0:1, q * 512:(q + 1) * 512] for q in range(4)]
            sy4 = [syr[0:1, (q % 2) * 512:(q % 2 + 1) * 512] for q in range(4)]
            # Cxy: cols 0:1024 use uxA, 1024:2048 use uxB (128-wide slice,
            # packed separately at 384:512 since 128:256 now holds uxB1|uxB2)
            uxB_full = ur[0:1, 384:512]
            jobs = [
                (c1a, P,
                 [(uxA, sy4[0]), (uxA, sy4[1]),
                  (uxB_full, sy4[2]), (uxB_full, sy4[3])], 0),
                (c1b, 64,
                 [(uyA, sy4[0]), (uyA, sy4[1]),
                  (uyB, sy4[2]), (uyB, sy4[3])], 1),
                (c2a, P, [(uxA, sx4[q]) for q in range(4)], 2),
                (c2b1, 96, [(uxB1, sx4[q]) for q in range(4)], 3),
                (c2b2, 32, [(uxB2, sx4[q]) for q in range(4)], 4),
            ]

            with tc.tile_pool(name="psum_c", bufs=2, space="PSUM") as psum_c:
                for (src, rows, uvs, col) in jobs:
                    corr = psum_c.tile([P, DX], f32, tag="corr")
                    for q, (u, v) in enumerate(uvs):
                        nc.tensor.matmul(corr[0:rows, q * 512:(q + 1) * 512],
                                         u, v, start=True, stop=True)
                    d = spill.tile([P, DX], bf16, tag="d", bufs=3)
                    nc.vector.tensor_sub(d[0:rows, :], src[0:rows, :],
                                         corr[0:rows, :])
                    sq = spill.tile([P, DX], f32, tag="sq", bufs=3)
                    nc.vector.tensor_mul(sq[0:rows, :], d[0:rows, :],
                                         d[0:rows, :])
                    nc.vector.tensor_reduce(
                        out=acc[0:rows, col:col + 1], in_=sq[0:rows, :],
                        axis=mybir.AxisListType.X, op=mybir.AluOpType.add)

            nc.sync.dma_start(out[:], acc[:])

    nc.compile()
    return nc


def _get_compiled():
    global _COMPILED
    if _COMPILED is None:
        _COMPILED = _build()
    return _COMPILED


def _run(x, y, trace=False):
    import ml_dtypes
    from concourse import bass_utils
    nc = _get_compiled()
    xb = np.ascontiguousarray(np.asarray(x)).astype(ml_dtypes.bfloat16)
    yb = np.ascontiguousarray(np.asarray(y)).astype(ml_dtypes.bfloat16)
    in_maps = [{"x": xb[r * NS:(r + 1) * NS], "y": yb[r * NS:(r + 1) * NS]}
               for r in range(N_CORES)]
    res = bass_utils.run_bass_kernel_spmd(
        nc, in_maps, core_ids=list(range(N_CORES)), trace=trace)
    hxy = hxx = hyy = 0.0
    for r in range(N_CORES):
        p = np.asarray(res.results[r]["partials"], dtype=np.float64)
        hxy += p[:, 0:1].sum()
        hyy += p[:, 1:2].sum()
        hxx += p[:, 2:5].sum()
    val = np.float32(hxy / (np.sqrt(hxx * hyy) + 1e-8))
    return np.asarray(val, dtype=np.float32), res


def kernel(x, y):
    val, _ = _run(x, y, trace=False)
    return val


# revision 16
# speedup vs baseline: 1.0364x; 1.0364x over previous
"""CKA (centered kernel alignment) on 8 Trainium2 NeuronCores.

Math: for G = x @ x.T, centering H G H (H = I - 11^T/n) satisfies
H G H = (Hx)(Hx)^T, so with xc = x - colmean(x):
    (K * L).sum() = ||xc^T @ yc||_F^2
and xc^T yc = x^T y - (1/n) sx sy^T  (sx/sy = column sums).
So CKA reduces to small feature-covariance matmuls instead of
8192x8192 Gram matrices (~120 GFLOP instead of ~412 + 536MB of traffic).

Sharding: rows (n) split across 8 cores. Each core computes partial
covariances (contraction over its 1024 rows) in bf16 (validated:
rel-err ~1e-3 on the final scalar incl. the bf16 CCE ring), packs
partials + column-sum partials into DRAM buffers, ReduceScatters them,
applies the exact rank-1 centering correction to its reduced chunk,
squares and reduces. Host sums 8 tiny [128,16] partials and applies the
final scalar formula.

Phase order is chosen so each ReduceScatter overlaps the next compute
phase. Collectives starve the kernel's own DMA queues (shared SDMA
hardware), so the spill pool is sized to hold an entire phase of
PSUM spills in SBUF — the PE keeps streaming while spill DMAs crawl
during a collective and burst afterwards:

  s-sums -> Cxx h1 -> [RS(bufA) || Cxy+Cyy] -> [RS(buf1) || Cxx h2]
         -> RS(bufB) -> center/square/reduce

Chunk layouts (per chunk c of 8):
  buf1 chunk (195 rows x 2048, bf16):
    rows   0..127 : Cxy[128c:+128, 0:1024] | Cxy[1024+128c:+128, 0:1024]
    rows 128..191 : Cyy[64c:+64, 0:1024]   | Cyy[512+64c:+64, 0:1024]
    row  192      : sx (full 2048 col-sums of x, replicated per chunk)
    row  193      : sy (full 1024 col-sums of y) | junk
    row  194      : packed u = s/n slices for THIS chunk's rows:
                    [sx[128c:+128]/n | sx[1024+128c:+128]/n
                     | sy[64c:+64]/n | sy[512+64c:+64]/n | junk]
  bufA chunk (128 rows): Cxx[128c:+128, :]       (Cxx rows 0..1023)
  bufB chunk (128 rows): Cxx[1024+128c:+128, :]  (Cxx rows 1024..2047)

Replicating the per-chunk s-slices inside each chunk keeps the SPMD
program rank-uniform: every core reads its u/v vectors at the same
static offsets of its own reduced chunk.
"""

import numpy as np

N_CORES = 8
N = 8192
NS = N // N_CORES      # 1024 rows per core
DX = 2048
DY = 1024
P = 128
KT = NS // P           # 8 contraction tiles per core
INV_N = 1.0 / N
C1R = 195              # buf1 rows per chunk

_COMPILED = None


def _build():
    import concourse.bacc as bacc
    import concourse.mybir as mybir
    import concourse.tile as tile

    f32 = mybir.dt.float32
    bf16 = mybir.dt.bfloat16

    nc = bacc.Bacc("TRN2", target_bir_lowering=False, debug=False,
                   num_devices=N_CORES)
    x = nc.dram_tensor("x", [NS, DX], bf16, kind="ExternalInput")
    y = nc.dram_tensor("y", [NS, DY], bf16, kind="ExternalInput")
    out = nc.dram_tensor("partials", [P, 20], f32, kind="ExternalOutput")

    rg = [list(range(N_CORES))]

    with tile.TileContext(nc) as tc:
        with (
            tc.tile_pool(name="persist", bufs=1) as persist,
            tc.tile_pool(name="spill", bufs=4) as spill,
            tc.tile_pool(name="dram", bufs=1, space="DRAM") as dram,
        ):
            # ---------------- load (already bf16) ----------------
            xb = persist.tile([P, KT, DX], bf16)
            yb = persist.tile([P, KT, DY], bf16)
            for k in range(KT):
                nc.sync.dma_start(xb[:, k, :], x[k * P:(k + 1) * P, :])
            for k in range(KT):
                nc.sync.dma_start(yb[:, k, :], y[k * P:(k + 1) * P, :])

            # DRAM buffers for the collectives
            buf1 = dram.tile([C1R * N_CORES, DX], bf16)
            bufA = dram.tile([1024, DX], bf16)
            bufB1 = dram.tile([768, DX], bf16)
            bufB2 = dram.tile([256, DX], bf16)
            ch1 = dram.tile([C1R, DX], bf16)
            chA = dram.tile([P, DX], bf16)
            chB1 = dram.tile([96, DX], bf16)
            chB2 = dram.tile([32, DX], bf16)
            scr_sx = dram.tile([1, DX], bf16)
            scr_sy = dram.tile([1, DY], bf16)
            scr_ux = dram.tile([1, DX], bf16)
            scr_uy = dram.tile([1, DY], bf16)
            bv = buf1[:].rearrange("(c r) w -> c r w", r=C1R)

            # ---------------- column sums (ones-matmul) ----------------
            ones = persist.tile([P, 1], bf16)
            nc.vector.memset(ones[:], 1.0)
            with tc.tile_pool(name="psum_s", bufs=1, space="PSUM") as psum_s:
                ps_sx = psum_s.tile([1, DX], f32)
                ps_sy = psum_s.tile([1, DY], f32)
                for k in range(KT):
                    for j in range(DX // 512):
                        nc.tensor.matmul(ps_sx[0:1, j * 512:(j + 1) * 512],
                                         ones[:], xb[:, k, j * 512:(j + 1) * 512],
                                         start=(k == 0), stop=(k == KT - 1))
                    for j in range(DY // 512):
                        nc.tensor.matmul(ps_sy[0:1, j * 512:(j + 1) * 512],
                                         ones[:], yb[:, k, j * 512:(j + 1) * 512],
                                         start=(k == 0), stop=(k == KT - 1))
                sx_sb = persist.tile([1, DX], bf16)
                sy_sb = persist.tile([1, DY], bf16)
                ux_sb = persist.tile([1, DX], bf16)
                uy_sb = persist.tile([1, DY], bf16)
                nc.scalar.copy(sx_sb[:], ps_sx[:])
                nc.scalar.copy(sy_sb[:], ps_sy[:])
                nc.scalar.mul(ux_sb[:], ps_sx[:], INV_N)
                nc.scalar.mul(uy_sb[:], ps_sy[:], INV_N)

            # s vectors -> DRAM scratch -> scatter into buf1 rows
            nc.sync.dma_start(scr_sx[:], sx_sb[:])
            nc.sync.dma_start(scr_sy[:], sy_sb[:])
            nc.sync.dma_start(scr_ux[:], ux_sb[:])
            nc.sync.dma_start(scr_uy[:], uy_sb[:])
            for c in range(N_CORES):
                nc.sync.dma_start(bv[c, 192, :], scr_sx[0, :])
                nc.sync.dma_start(bv[c, 193, 0:DY], scr_sy[0, :])
            nc.sync.dma_start(
                bv[:, 194, 0:128],
                scr_ux[0:1, 0:1024].rearrange("a (c k) -> (a c) k", k=128))
            nc.sync.dma_start(
                bv[:, 194, 128:224],
                scr_ux[0:1, 1024:1792].rearrange("a (c k) -> (a c) k", k=96))
            nc.sync.dma_start(
                bv[:, 194, 224:256],
                scr_ux[0:1, 1792:2048].rearrange("a (c k) -> (a c) k", k=32))
            nc.sync.dma_start(
                bv[:, 194, 256:320],
                scr_uy[0:1, 0:512].rearrange("a (c k) -> (a c) k", k=64))
            nc.sync.dma_start(
                bv[:, 194, 320:384],
                scr_uy[0:1, 512:1024].rearrange("a (c k) -> (a c) k", k=64))

            with tc.tile_pool(name="psum_mm", bufs=8, space="PSUM") as psum_mm:

                def cxx_half(half, dsts):
                    for mh in range(8):
                        m = half * 8 + mh
                        pss = [psum_mm.tile([P, 512], f32, tag="ps", name="ps")
                               for _ in range(4)]
                        for k in range(KT):
                            for n4 in range(4):
                                nc.tensor.matmul(
                                    pss[n4][:], xb[:, k, m * P:(m + 1) * P],
                                    xb[:, k, n4 * 512:(n4 + 1) * 512],
                                    start=(k == 0), stop=(k == KT - 1))
                        if mh < 6:
                            dst, r0 = dsts[0], mh * P
                        else:
                            dst, r0 = dsts[1], (mh - 6) * P
                        for n4 in range(4):
                            st = spill.tile([P, 512], bf16, tag="st",
                                            name="st", bufs=56)
                            nc.vector.tensor_copy(st[:], pss[n4][:])
                            nc.sync.dma_start(
                                dst[r0:r0 + P,
                                    n4 * 512:(n4 + 1) * 512], st[:])

                # ---- Cxx first half -> bufA, then its ReduceScatter ----
                cxx_half(0, (bufA, bufA[768:1024, :]))
                nc.gpsimd.collective_compute(
                    "ReduceScatter", mybir.AluOpType.add, replica_groups=rg,
                    ins=[bufA[:]], outs=[chA[:]])

                # ---- Cxy (overlaps RS(bufA)) ----
                for m in range(DX // P):
                    pss = [psum_mm.tile([P, 512], f32, tag="ps", name="ps")
                           for _ in range(2)]
                    for k in range(KT):
                        for n2 in range(2):
                            nc.tensor.matmul(
                                pss[n2][:], xb[:, k, m * P:(m + 1) * P],
                                yb[:, k, n2 * 512:(n2 + 1) * 512],
                                start=(k == 0), stop=(k == KT - 1))
                    c, col0 = (m, 0) if m < 8 else (m - 8, 1024)
                    for n2 in range(2):
                        st = spill.tile([P, 512], bf16, tag="st",
                                        name="st", bufs=56)
                        nc.vector.tensor_copy(st[:], pss[n2][:])
                        nc.sync.dma_start(
                            bv[c, 0:P, col0 + n2 * 512:col0 + (n2 + 1) * 512],
                            st[:])

                # ---- Cyy ----
                for m in range(DY // P):
                    pss = [psum_mm.tile([P, 512], f32, tag="ps", name="ps")
                           for _ in range(2)]
                    for k in range(KT):
                        for n2 in range(2):
                            nc.tensor.matmul(
                                pss[n2][:], yb[:, k, m * P:(m + 1) * P],
                                yb[:, k, n2 * 512:(n2 + 1) * 512],
                                start=(k == 0), stop=(k == KT - 1))
                    for n2 in range(2):
                        st = spill.tile([P, 512], bf16, tag="st",
                                        name="st", bufs=56)
                        nc.vector.tensor_copy(st[:], pss[n2][:])
                        for h in range(2):
                            mm = m if m < 4 else m - 4
                            c = 2 * mm + h
                            col0 = (0 if m < 4 else 1024) + n2 * 512
                            nc.sync.dma_start(
                                bv[c, 128:192, col0:col0 + 512],
                                st[h * 64:(h + 1) * 64, :])

                # ---- ReduceScatter buf1 (overlaps Cxx h2) ----
                nc.gpsimd.collective_compute(
                    "ReduceScatter", mybir.AluOpType.add, replica_groups=rg,
                    ins=[buf1[:]], outs=[ch1[:]])

                # ---- Cxx second half -> bufB, then its ReduceScatter ----
                cxx_half(1, (bufB1, bufB2))
                nc.gpsimd.collective_compute(
                    "ReduceScatter", mybir.AluOpType.add, replica_groups=rg,
                    ins=[bufB1[:]], outs=[chB1[:]])
                nc.gpsimd.collective_compute(
                    "ReduceScatter", mybir.AluOpType.add, replica_groups=rg,
                    ins=[bufB2[:]], outs=[chB2[:]])

            # ------------- stage 2: center, square, reduce -------------
            sxr = persist.tile([1, DX], bf16)
            nc.sync.dma_start(sxr[:], ch1[192:193, :])
            syr = persist.tile([1, DY], bf16)
            nc.sync.dma_start(syr[:], ch1[193:194, 0:DY])
            ur = persist.tile([1, 384], bf16)
            nc.sync.dma_start(ur[:], ch1[194:195, 0:384])
            c1a = persist.tile([P, DX], bf16)
            nc.sync.dma_start(c1a[:], ch1[0:P, :])
            c1b = persist.tile([64, DX], bf16)
            nc.sync.dma_start(c1b[:], ch1[P:192, :])
            c2a = persist.tile([P, DX], bf16)
            nc.sync.dma_start(c2a[:], chA[:])
            c2b1 = persist.tile([96, DX], bf16)
            nc.sync.dma_start(c2b1[:], chB1[:])
            c2b2 = persist.tile([32, DX], bf16)
            nc.sync.dma_start(c2b2[:], chB2[:])

            acc = persist.tile([P, 20], f32)
            nc.vector.memset(acc[:], 0.0)

            # jobs: (chunk tile, rows, col-chunk n4, u slice, v slice, col)
            # u offsets in ur: uxA@0, uxB@128, uyA@256, uyB@320
            jobs = []
            for n4 in range(4):   # Cxy: cols 0:1024 use uxA, 1024:2048 uxB
                u = ur[0:1, 0:128] if n4 < 2 else ur[0:1, 128:256]
                v = syr[0:1, (n4 % 2) * 512:(n4 % 2 + 1) * 512]
                jobs.append((c1a, P, n4, u, v, n4))
            for n4 in range(4):   # Cyy: uyA / uyB; v=sy
                u = ur[0:1, 256:320] if n4 < 2 else ur[0:1, 320:384]
                v = syr[0:1, (n4 % 2) * 512:(n4 % 2 + 1) * 512]
                jobs.append((c1b, 64, n4, u, v, 4 + n4))
            for n4 in range(4):   # Cxx rows 0..1023: uxA; v=sx
                jobs.append((c2a, P, n4, ur[0:1, 0:128],
                             sxr[0:1, n4 * 512:(n4 + 1) * 512], 8 + n4))
            for n4 in range(4):   # Cxx rows 1024..1791: uxB1; v=sx
                jobs.append((c2b1, 96, n4, ur[0:1, 128:224],
                             sxr[0:1, n4 * 512:(n4 + 1) * 512], 12 + n4))
            for n4 in range(4):   # Cxx rows 1792..2047: uxB2; v=sx
                jobs.append((c2b2, 32, n4, ur[0:1, 224:256],
                             sxr[0:1, n4 * 512:(n4 + 1) * 512], 16 + n4))

            with tc.tile_pool(name="psum_c", bufs=4, space="PSUM") as psum_c:
                for (src, rows, n4, u, v, col) in jobs:
                    corr = psum_c.tile([P, 512], f32, tag="corr")
                    nc.tensor.matmul(corr[0:rows, :], u, v,
                                     start=True, stop=True)
                    d = spill.tile([P, 512], bf16, tag="d", bufs=8)
                    nc.vector.tensor_sub(
                        d[0:rows, :], src[0:rows, n4 * 512:(n4 + 1) * 512],
                        corr[0:rows, :])
                    sq = spill.tile([P, 512], f32, tag="sq", bufs=8)
                    nc.vector.tensor_mul(sq[0:rows, :], d[0:rows, :],
                                         d[0:rows, :])
                    nc.vector.tensor_reduce(
                        out=acc[0:rows, col:col + 1], in_=sq[0:rows, :],
                        axis=mybir.AxisListType.X, op=mybir.AluOpType.add)

            nc.sync.dma_start(out[:], acc[:])

    nc.compile()
    return nc


def _get_compiled():
    global _COMPILED
    if _COMPILED is None:
        _COMPILED = _build()
    return _COMPILED


def _run(x, y, trace=False):
    import ml_dtypes
    from concourse import bass_utils
    nc = _get_compiled()
    xb = np.ascontiguousarray(np.asarray(x)).astype(ml_dtypes.bfloat16)
    yb = np.ascontiguousarray(np.asarray(y)).astype(ml_dtypes.bfloat16)
    in_maps = [{"x": xb[r * NS:(r + 1) * NS], "y": yb[r * NS:(r + 1) * NS]}
               for r in range(N_CORES)]
    res = bass_utils.run_bass_kernel_spmd(
        nc, in_maps, core_ids=list(range(N_CORES)), trace=trace)
    hxy = hxx = hyy = 0.0
    for r in range(N_CORES):
        p = np.asarray(res.results[r]["partials"], dtype=np.float64)
        hxy += p[:, 0:4].sum()
        hyy += p[:, 4:8].sum()
        hxx += p[:, 8:20].sum()
    val = np.float32(hxy / (np.sqrt(hxx * hyy) + 1e-8))
    return np.asarray(val, dtype=np.float32), res


def kernel(x, y):
    val, _ = _run(x, y, trace=False)
    return val


# revision 17
# speedup vs baseline: 1.1173x; 1.0781x over previous
"""CKA (centered kernel alignment) on 8 Trainium2 NeuronCores.

Math: for G = x @ x.T, centering H G H (H = I - 11^T/n) satisfies
H G H = (Hx)(Hx)^T, so with xc = x - colmean(x):
    (K * L).sum() = ||xc^T @ yc||_F^2
and xc^T yc = x^T y - (1/n) sx sy^T  (sx/sy = column sums).
So CKA reduces to small feature-covariance matmuls instead of
8192x8192 Gram matrices (~120 GFLOP instead of ~412 + 536MB of traffic).

Sharding: rows (n) split across 8 cores. Each core computes partial
covariances (contraction over its 1024 rows) in bf16 (validated:
rel-err ~1e-3 on the final scalar incl. the bf16 CCE ring), packs
partials + column-sum partials into DRAM buffers, ReduceScatters them,
applies the exact rank-1 centering correction to its reduced chunk,
squares and reduces. Host sums 8 tiny [128,16] partials and applies the
final scalar formula.

Phase order is chosen so each ReduceScatter overlaps the next compute
phase. Collectives starve the kernel's own DMA queues (shared SDMA
hardware), so the spill pool is sized to hold an entire phase of
PSUM spills in SBUF — the PE keeps streaming while spill DMAs crawl
during a collective and burst afterwards:

  s-sums -> Cxx h1 -> [RS(bufA) || Cxy+Cyy] -> [RS(buf1) || Cxx h2]
         -> RS(bufB) -> center/square/reduce

Chunk layouts (per chunk c of 8):
  buf1 chunk (195 rows x 2048, bf16):
    rows   0..127 : Cxy[128c:+128, 0:1024] | Cxy[1024+128c:+128, 0:1024]
    rows 128..191 : Cyy[64c:+64, 0:1024]   | Cyy[512+64c:+64, 0:1024]
    row  192      : sx (full 2048 col-sums of x, replicated per chunk)
    row  193      : sy (full 1024 col-sums of y) | junk
    row  194      : packed u = s/n slices for THIS chunk's rows:
                    [sx[128c:+128]/n | sx[1024+128c:+128]/n
                     | sy[64c:+64]/n | sy[512+64c:+64]/n | junk]
  bufA chunk (128 rows): Cxx[128c:+128, :]       (Cxx rows 0..1023)
  bufB chunk (128 rows): Cxx[1024+128c:+128, :]  (Cxx rows 1024..2047)

Replicating the per-chunk s-slices inside each chunk keeps the SPMD
program rank-uniform: every core reads its u/v vectors at the same
static offsets of its own reduced chunk.
"""

import numpy as np

N_CORES = 8
N = 8192
NS = N // N_CORES      # 1024 rows per core
DX = 2048
DY = 1024
P = 128
KT = NS // P           # 8 contraction tiles per core
INV_N = 1.0 / N
C1R = 195              # buf1 rows per chunk

_COMPILED = None


def _build():
    import concourse.bacc as bacc
    import concourse.mybir as mybir
    import concourse.tile as tile

    f32 = mybir.dt.float32
    bf16 = mybir.dt.bfloat16

    nc = bacc.Bacc("TRN2", target_bir_lowering=False, debug=False,
                   num_devices=N_CORES)
    x = nc.dram_tensor("x", [NS, DX], bf16, kind="ExternalInput")
    y = nc.dram_tensor("y", [NS, DY], bf16, kind="ExternalInput")
    out = nc.dram_tensor("partials", [P, 20], f32, kind="ExternalOutput")

    rg = [list(range(N_CORES))]

    with tile.TileContext(nc) as tc:
        with (
            tc.tile_pool(name="persist", bufs=1) as persist,
            tc.tile_pool(name="spill", bufs=4) as spill,
            tc.tile_pool(name="dram", bufs=1, space="DRAM") as dram,
        ):
            # ---------------- load (already bf16) ----------------
            xb = persist.tile([P, KT, DX], bf16)
            yb = persist.tile([P, KT, DY], bf16)
            for k in range(KT):
                nc.sync.dma_start(xb[:, k, :], x[k * P:(k + 1) * P, :])
            for k in range(KT):
                nc.sync.dma_start(yb[:, k, :], y[k * P:(k + 1) * P, :])

            # DRAM buffers for the collectives
            bufC = dram.tile([1024, DX], bf16)
            bufD = dram.tile([67 * N_CORES, DX], bf16)
            bufA = dram.tile([1024, DX], bf16)
            bufB1 = dram.tile([768, DX], bf16)
            bufB2 = dram.tile([256, DX], bf16)
            chC = dram.tile([P, DX], bf16)
            chD = dram.tile([67, DX], bf16)
            chA = dram.tile([P, DX], bf16)
            chB1 = dram.tile([96, DX], bf16)
            chB2 = dram.tile([32, DX], bf16)
            scr_sx = dram.tile([1, DX], bf16)
            scr_sy = dram.tile([1, DY], bf16)
            scr_ux = dram.tile([1, DX], bf16)
            scr_uy = dram.tile([1, DY], bf16)
            bdv = bufD[:].rearrange("(c r) w -> c r w", r=67)

            # ---------------- column sums (ones-matmul) ----------------
            ones = persist.tile([P, 1], bf16)
            nc.vector.memset(ones[:], 1.0)
            with tc.tile_pool(name="psum_s", bufs=1, space="PSUM") as psum_s:
                ps_sx = psum_s.tile([1, DX], f32)
                ps_sy = psum_s.tile([1, DY], f32)
                for k in range(KT):
                    for j in range(DX // 512):
                        nc.tensor.matmul(ps_sx[0:1, j * 512:(j + 1) * 512],
                                         ones[:], xb[:, k, j * 512:(j + 1) * 512],
                                         start=(k == 0), stop=(k == KT - 1))
                    for j in range(DY // 512):
                        nc.tensor.matmul(ps_sy[0:1, j * 512:(j + 1) * 512],
                                         ones[:], yb[:, k, j * 512:(j + 1) * 512],
                                         start=(k == 0), stop=(k == KT - 1))
                sx_sb = persist.tile([1, DX], bf16)
                sy_sb = persist.tile([1, DY], bf16)
                ux_sb = persist.tile([1, DX], bf16)
                uy_sb = persist.tile([1, DY], bf16)
                nc.scalar.copy(sx_sb[:], ps_sx[:])
                nc.scalar.copy(sy_sb[:], ps_sy[:])
                nc.scalar.mul(ux_sb[:], ps_sx[:], INV_N)
                nc.scalar.mul(uy_sb[:], ps_sy[:], INV_N)

            # s vectors -> DRAM scratch -> scatter into buf1 rows
            nc.sync.dma_start(scr_sx[:], sx_sb[:])
            nc.sync.dma_start(scr_sy[:], sy_sb[:])
            nc.sync.dma_start(scr_ux[:], ux_sb[:])
            nc.sync.dma_start(scr_uy[:], uy_sb[:])
            for c in range(N_CORES):
                nc.sync.dma_start(bdv[c, 64, :], scr_sx[0, :])
                nc.sync.dma_start(bdv[c, 65, 0:DY], scr_sy[0, :])
            nc.sync.dma_start(
                bdv[:, 66, 0:128],
                scr_ux[0:1, 0:1024].rearrange("a (c k) -> (a c) k", k=128))
            nc.sync.dma_start(
                bdv[:, 66, 128:224],
                scr_ux[0:1, 1024:1792].rearrange("a (c k) -> (a c) k", k=96))
            nc.sync.dma_start(
                bdv[:, 66, 224:256],
                scr_ux[0:1, 1792:2048].rearrange("a (c k) -> (a c) k", k=32))
            nc.sync.dma_start(
                bdv[:, 66, 256:320],
                scr_uy[0:1, 0:512].rearrange("a (c k) -> (a c) k", k=64))
            nc.sync.dma_start(
                bdv[:, 66, 320:384],
                scr_uy[0:1, 512:1024].rearrange("a (c k) -> (a c) k", k=64))
            nc.sync.dma_start(
                bdv[:, 66, 384:512],
                scr_ux[0:1, 1024:2048].rearrange("a (c k) -> (a c) k", k=128))

            with tc.tile_pool(name="psum_mm", bufs=8, space="PSUM") as psum_mm:

                def cxx_half(half, dsts):
                    for mh in range(8):
                        m = half * 8 + mh
                        pss = [psum_mm.tile([P, 512], f32, tag="ps", name="ps")
                               for _ in range(4)]
                        for k in range(KT):
                            for n4 in range(4):
                                nc.tensor.matmul(
                                    pss[n4][:], xb[:, k, m * P:(m + 1) * P],
                                    xb[:, k, n4 * 512:(n4 + 1) * 512],
                                    start=(k == 0), stop=(k == KT - 1))
                        if mh < 6:
                            dst, r0 = dsts[0], mh * P
                        else:
                            dst, r0 = dsts[1], (mh - 6) * P
                        for n4 in range(4):
                            st = spill.tile([P, 512], bf16, tag="st",
                                            name="st", bufs=56)
                            nc.vector.tensor_copy(st[:], pss[n4][:])
                            nc.sync.dma_start(
                                dst[r0:r0 + P,
                                    n4 * 512:(n4 + 1) * 512], st[:])

                # ---- Cxx first half -> bufA, then its ReduceScatter ----
                cxx_half(0, (bufA, bufA[768:1024, :]))
                nc.gpsimd.collective_compute(
                    "ReduceScatter", mybir.AluOpType.add, replica_groups=rg,
                    ins=[bufA[:]], outs=[chA[:]])

                # ---- Cxy (overlaps RS(bufA)) ----
                for m in range(DX // P):
                    pss = [psum_mm.tile([P, 512], f32, tag="ps", name="ps")
                           for _ in range(2)]
                    for k in range(KT):
                        for n2 in range(2):
                            nc.tensor.matmul(
                                pss[n2][:], xb[:, k, m * P:(m + 1) * P],
                                yb[:, k, n2 * 512:(n2 + 1) * 512],
                                start=(k == 0), stop=(k == KT - 1))
                    c, col0 = (m, 0) if m < 8 else (m - 8, 1024)
                    for n2 in range(2):
                        st = spill.tile([P, 512], bf16, tag="st",
                                        name="st", bufs=56)
                        nc.vector.tensor_copy(st[:], pss[n2][:])
                        nc.sync.dma_start(
                            bufC[c * P:(c + 1) * P,
                                 col0 + n2 * 512:col0 + (n2 + 1) * 512],
                            st[:])

                nc.gpsimd.collective_compute(
                    "ReduceScatter", mybir.AluOpType.add, replica_groups=rg,
                    ins=[bufC[:]], outs=[chC[:]])

                # ---- Cyy ----
                for m in range(DY // P):
                    pss = [psum_mm.tile([P, 512], f32, tag="ps", name="ps")
                           for _ in range(2)]
                    for k in range(KT):
                        for n2 in range(2):
                            nc.tensor.matmul(
                                pss[n2][:], yb[:, k, m * P:(m + 1) * P],
                                yb[:, k, n2 * 512:(n2 + 1) * 512],
                                start=(k == 0), stop=(k == KT - 1))
                    for n2 in range(2):
                        st = spill.tile([P, 512], bf16, tag="st",
                                        name="st", bufs=56)
                        nc.vector.tensor_copy(st[:], pss[n2][:])
                        for h in range(2):
                            mm = m if m < 4 else m - 4
                            c = 2 * mm + h
                            col0 = (0 if m < 4 else 1024) + n2 * 512
                            nc.sync.dma_start(
                                bdv[c, 0:64, col0:col0 + 512],
                                st[h * 64:(h + 1) * 64, :])

                # ---- ReduceScatter bufD (overlaps Cxx h2) ----
                nc.gpsimd.collective_compute(
                    "ReduceScatter", mybir.AluOpType.add, replica_groups=rg,
                    ins=[bufD[:]], outs=[chD[:]])

                # ---- Cxx second half -> bufB, then its ReduceScatter ----
                cxx_half(1, (bufB1, bufB2))
                nc.gpsimd.collective_compute(
                    "ReduceScatter", mybir.AluOpType.add, replica_groups=rg,
                    ins=[bufB1[:]], outs=[chB1[:]])
                nc.gpsimd.collective_compute(
                    "ReduceScatter", mybir.AluOpType.add, replica_groups=rg,
                    ins=[bufB2[:]], outs=[chB2[:]])

            # ------------- stage 2: center, square, reduce -------------
            sxr = persist.tile([1, DX], bf16)
            nc.sync.dma_start(sxr[:], chD[64:65, :])
            syr = persist.tile([1, DY], bf16)
            nc.sync.dma_start(syr[:], chD[65:66, 0:DY])
            ur = persist.tile([1, 512], bf16)
            nc.sync.dma_start(ur[:], chD[66:67, 0:512])
            c1a = persist.tile([P, DX], bf16)
            nc.sync.dma_start(c1a[:], chC[:])
            c1b = persist.tile([64, DX], bf16)
            nc.sync.dma_start(c1b[:], chD[0:64, :])
            c2a = persist.tile([P, DX], bf16)
            nc.sync.dma_start(c2a[:], chA[:])
            c2b1 = persist.tile([96, DX], bf16)
            nc.sync.dma_start(c2b1[:], chB1[:])
            c2b2 = persist.tile([32, DX], bf16)
            nc.sync.dma_start(c2b2[:], chB2[:])

            acc = persist.tile([P, 20], f32)
            nc.vector.memset(acc[:], 0.0)

            # jobs: (chunk tile, rows, col-chunk n4, u slice, v slice, col)
            # u offsets in ur: uxA@0, uxB@128, uyA@256, uyB@320
            jobs = []
            for n4 in range(4):   # Cxy: cols 0:1024 use uxA, 1024:2048 uxB
                u = ur[0:1, 0:128] if n4 < 2 else ur[0:1, 384:512]
                v = syr[0:1, (n4 % 2) * 512:(n4 % 2 + 1) * 512]
                jobs.append((c1a, P, n4, u, v, n4))
            for n4 in range(4):   # Cyy: uyA / uyB; v=sy
                u = ur[0:1, 256:320] if n4 < 2 else ur[0:1, 320:384]
                v = syr[0:1, (n4 % 2) * 512:(n4 % 2 + 1) * 512]
                jobs.append((c1b, 64, n4, u, v, 4 + n4))
            for n4 in range(4):   # Cxx rows 0..1023: uxA; v=sx
                jobs.append((c2a, P, n4, ur[0:1, 0:128],
                             sxr[0:1, n4 * 512:(n4 + 1) * 512], 8 + n4))
            for n4 in range(4):   # Cxx rows 1024..1791: uxB1; v=sx
                jobs.append((c2b1, 96, n4, ur[0:1, 128:224],
                             sxr[0:1, n4 * 512:(n4 + 1) * 512], 12 + n4))
            for n4 in range(4):   # Cxx rows 1792..2047: uxB2; v=sx
                jobs.append((c2b2, 32, n4, ur[0:1, 224:256],
                             sxr[0:1, n4 * 512:(n4 + 1) * 512], 16 + n4))

            with tc.tile_pool(name="psum_c", bufs=4, space="PSUM") as psum_c:
                for (src, rows, n4, u, v, col) in jobs:
                    corr = psum_c.tile([P, 512], f32, tag="corr")
                    nc.tensor.matmul(corr[0:rows, :], u, v,
                                     start=True, stop=True)
                    d = spill.tile([P, 512], bf16, tag="d", bufs=8)
                    nc.vector.tensor_sub(
                        d[0:rows, :], src[0:rows, n4 * 512:(n4 + 1) * 512],
                        corr[0:rows, :])
                    sq = spill.tile([P, 512], f32, tag="sq", bufs=8)
                    nc.vector.tensor_mul(sq[0:rows, :], d[0:rows, :],
                                         d[0:rows, :])
                    nc.vector.tensor_reduce(
                        out=acc[0:rows, col:col + 1], in_=sq[0:rows, :],
                        axis=mybir.AxisListType.X, op=mybir.AluOpType.add)

            nc.sync.dma_start(out[:], acc[:])

    nc.compile()
    return nc


def _get_compiled():
    global _COMPILED
    if _COMPILED is None:
        _COMPILED = _build()
    return _COMPILED


def _run(x, y, trace=False):
    import ml_dtypes
    from concourse import bass_utils
    nc = _get_compiled()
    xb = np.ascontiguousarray(np.asarray(x)).astype(ml_dtypes.bfloat16)
    yb = np.ascontiguousarray(np.asarray(y)).astype(ml_dtypes.bfloat16)
    in_maps = [{"x": xb[r * NS:(r + 1) * NS], "y": yb[r * NS:(r + 1) * NS]}
               for r in range(N_CORES)]
    res = bass_utils.run_bass_kernel_spmd(
        nc, in_maps, core_ids=list(range(N_CORES)), trace=trace)
    hxy = hxx = hyy = 0.0
    for r in range(N_CORES):
        p = np.asarray(res.results[r]["partials"], dtype=np.float64)
        hxy += p[:, 0:4].sum()
        hyy += p[:, 4:8].sum()
        hxx += p[:, 8:20].sum()
    val = np.float32(hxy / (np.sqrt(hxx * hyy) + 1e-8))
    return np.asarray(val, dtype=np.float32), res


def kernel(x, y):
    val, _ = _run(x, y, trace=False)
    return val
